# revision 1
# baseline (speedup 1.0000x reference)
"""Trainium2 Bass kernel for nn_ExtractPatchesPositionLayer.

Reference semantics: per image b, bilinear-translate the (522,522,1) padded
object by t = -positions[b] (tfa.translate: out(y,x) = img(y+py, x+px),
zero fill outside), then center-crop 5px -> (512,512,1).

Because the shift is constant per image, floor/frac of the offset give an
integer window start (A,B) into the (zero-margin-padded) image plus four
constant bilinear corner weights. The whole bilinear then collapses into two
accumulating PE matmuls per 127-row chunk:

    psum[m, j] = sum_k Bv0[k, m] * W[k, j] + sum_k Bv1[k, m] * W[k, j+1]

with banded 128x127 matrices
    Bv0 = c00*I + c10*S,  Bv1 = c01*I + c11*S
    (I[k,m] = d_{k,m}, S[k,m] = d_{k,m+1};
     c00=(1-wy)(1-wx), c10=wy(1-wx), c01=(1-wy)wx, c11=wy*wx)

The per-image window is fetched with dynamic HWDGE DMAs: host-precomputed
flat element offsets (int32 data) are reg_load-ed into a small pool of
rotating SP registers and used as runtime AP offsets, so one SPMD program
serves all cores with no data-dependent immediates. (Indirect/gather DMA was
tried first but SWDGE lands every gather descriptor on DMA engine 0 —
1.4 ms serialized; dynamic HWDGE DMAs split across all 16 engines.)
Sharding: batch 256 -> 32 images x 8 cores, embarrassingly parallel,
no communication.
"""

from dataclasses import dataclass

import numpy as np

import concourse.bacc as bacc
import concourse.bass as bass
import concourse.mybir as mybir
import concourse.tile as tile
from concourse.bass_utils import run_bass_kernel_spmd


@dataclass(frozen=True)
class Cfg:
    bpc: int      # images per core
    n: int        # output height/width
    wpad: int     # padded input height/width (with zero margin)
    chunk: int    # output rows per matmul chunk (<=127)

    @property
    def win(self):  # window width loaded per chunk
        return self.n + 1

    @property
    def chunks(self):
        out = []
        r = 0
        while r < self.n:
            nr = min(self.chunk, self.n - r)
            out.append((r, nr))
            r += nr
        return out

    @property
    def nbig(self):
        return sum(1 for _, nr in self.chunks if nr == self.chunk)

    @property
    def rem(self):  # (row0, nrows) of the non-uniform trailing chunk, if any
        r = self.chunks[self.nbig:]
        assert len(r) <= 1
        return r[0] if r else None


def build_nc(cfg: Cfg) -> bass.Bass:
    BPC, N, WPAD, WIN = cfg.bpc, cfg.n, cfg.wpad, cfg.win
    CH = cfg.chunk
    nbig = cfg.nbig
    rem = cfg.rem
    P = CH + 1
    PS = (rem[1] + 1) if rem else 1  # partitions of the remainder gather
    TOT = BPC * WPAD * WPAD
    f32 = mybir.dt.float32
    i32 = mybir.dt.int32

    nc = bacc.Bacc("TRN2", target_bir_lowering=False, debug=False)
    x_d = nc.declare_dram_parameter("x", [BPC, WPAD, WPAD], f32, isOutput=False)
    offs_d = nc.declare_dram_parameter("offs", [1, BPC * 2], i32, isOutput=False)
    wmat_d = nc.declare_dram_parameter("wmat", [BPC, 128, 4], f32, isOutput=False)
    dmat_d = nc.declare_dram_parameter("dmat", [128, 2 * CH], f32, isOutput=False)
    y_d = nc.declare_dram_parameter("y", [BPC, N, N], f32, isOutput=True)

    with tile.TileContext(nc) as tc:
        with (
            tc.tile_pool(name="const", bufs=1) as constp,
            tc.tile_pool(name="bmat", bufs=3) as bmatp,
            tc.tile_pool(name="win", bufs=3) as winp,
            tc.tile_pool(name="outp", bufs=3) as outp,
            tc.tile_pool(name="ps", bufs=6, space="PSUM") as psp,
        ):
            dmat_sb = constp.tile([128, 2 * CH], f32, tag="dmat")
            nc.sync.dma_start(dmat_sb[:], dmat_d[:, :])
            wmat_sb = constp.tile([128, BPC * 4], f32, tag="wmat")
            nc.sync.dma_start(
                wmat_sb[:].rearrange("p (i q) -> p i q", q=4),
                wmat_d[:, :, :].transpose([1, 0, 2]),
            )
            offs_sb = constp.tile([1, BPC * 2], i32, tag="offs")
            nc.sync.dma_start(offs_sb[:], offs_d[:, :])
            d0 = dmat_sb[:, 0:CH]
            d1 = dmat_sb[:, CH:2 * CH]

            # two register pools, one per HWDGE ring (SP + ACT); alternating
            # the big window loads across both rings doubles descriptor-gen
            # fan-out (a single dynamic DMA's descriptors serialize on one
            # DMA engine otherwise)
            off_max = TOT - 1
            pools = []
            for eng_t, eng in ((mybir.EngineType.SP, nc.sync),
                               (mybir.EngineType.Activation, nc.scalar)):
                regs = [nc.alloc_register(eng_t, f"dynoff_{eng_t}_{k}")
                        for k in range(min(8, 2 * BPC))]
                svs = [nc.snap(r, donate=True, min_val=0, max_val=off_max)
                       for r in regs]
                pools.append((eng, regs, svs))

            for i in range(BPC):
                # per-image banded matrices Bv0, Bv1 on DVE
                b0 = bmatp.tile([128, CH], f32, tag="b0")
                b1 = bmatp.tile([128, CH], f32, tag="b1")
                t0 = bmatp.tile([128, CH], f32, tag="t0")
                t1 = bmatp.tile([128, CH], f32, tag="t1")
                c00 = wmat_sb[:, 4 * i + 0: 4 * i + 1]
                c10 = wmat_sb[:, 4 * i + 1: 4 * i + 2]
                c01 = wmat_sb[:, 4 * i + 2: 4 * i + 3]
                c11 = wmat_sb[:, 4 * i + 3: 4 * i + 4]
                nc.scalar.mul(t0[:], d1, c10)
                nc.scalar.mul(b0[:], d0, c00)
                nc.vector.tensor_add(b0[:], b0[:], t0[:])
                nc.scalar.mul(t1[:], d1, c11)
                nc.scalar.mul(b1[:], d0, c01)
                nc.vector.tensor_add(b1[:], b1[:], t1[:])

                # dynamic flat element offsets, host-precomputed per DMA;
                # one strided DMA loads all uniform chunks:
                # wt_big[p, c, w] = x.flat[off_big + (c*CH + p)*WPAD + w]
                eng, regs, svs = pools[i % 2]
                nreg = len(regs)
                kb = (2 * i) % nreg
                eng.reg_load(regs[kb], offs_sb[0:1, 2 * i: 2 * i + 1])
                wt_big = winp.tile([P, nbig * WIN], f32, tag="wt_big")
                eng.dma_start(
                    wt_big[:].rearrange("p (c w) -> p c w", w=WIN),
                    bass.AP(x_d, svs[kb],
                            [[WPAD, P], [CH * WPAD, nbig], [1, WIN]]),
                )
                if rem:
                    ks = (2 * i + 1) % nreg
                    eng.reg_load(regs[ks], offs_sb[0:1, 2 * i + 1: 2 * i + 2])
                    wt_s = winp.tile([PS, WIN], f32, tag="wt_s")
                    eng.dma_start(
                        wt_s[:],
                        bass.AP(x_d, svs[ks], [[WPAD, PS], [1, WIN]]),
                    )

                ob_big = outp.tile([CH, nbig * N], f32, tag="ob_big")
                for c in range(nbig):
                    ps = psp.tile([CH, N], f32, tag="ps")
                    rhs0 = wt_big[:P, c * WIN: c * WIN + N]
                    rhs1 = wt_big[:P, c * WIN + 1: c * WIN + 1 + N]
                    nc.tensor.matmul(out=ps[:], lhsT=b0[:P, :], rhs=rhs0,
                                     start=True, stop=False)
                    nc.tensor.matmul(out=ps[:], lhsT=b1[:P, :], rhs=rhs1,
                                     start=False, stop=True)
                    nc.scalar.copy(ob_big[:, c * N:(c + 1) * N], ps[:])
                # store the uniform chunks with one strided DMA:
                # y[i, c*CH + m, j] = ob_big[m, c*N + j]
                nc.sync.dma_start(
                    bass.AP(y_d, i * (N * N),
                            [[N, CH], [CH * N, nbig], [1, N]]),
                    ob_big[:].rearrange("p (c w) -> p c w", w=N),
                )
                if rem:
                    r0r, nrr = rem
                    ps_s = psp.tile([CH, N], f32, tag="ps")
                    ob_s = outp.tile([max(nrr, 1), N], f32, tag="ob_s")
                    nc.tensor.matmul(out=ps_s[:nrr, :],
                                     lhsT=b0[:nrr + 1, :nrr],
                                     rhs=wt_s[:nrr + 1, 0:N],
                                     start=True, stop=False)
                    nc.tensor.matmul(out=ps_s[:nrr, :],
                                     lhsT=b1[:nrr + 1, :nrr],
                                     rhs=wt_s[:nrr + 1, 1:N + 1],
                                     start=False, stop=True)
                    nc.scalar.copy(ob_s[:nrr, :], ps_s[:nrr, :])
                    nc.sync.dma_start(y_d[i, r0r:r0r + nrr, :], ob_s[:nrr, :])
    nc.compile()
    return nc


def host_prep(padded: np.ndarray, positions: np.ndarray, n_cores: int, chunk: int):
    """Shard + build metadata. padded: (B, npad, npad) f32, positions: (B, 2)."""
    B, npad, _ = padded.shape
    n = npad - 10
    bpc = B // n_cores
    win = n + 1

    px = positions[:, 0].astype(np.float32)
    py = positions[:, 1].astype(np.float32)
    fy = np.floor(py)
    fx = np.floor(px)
    ay = (5 + fy).astype(np.int64)
    ax = (5 + fx).astype(np.int64)
    wy = (py - fy).astype(np.float32)
    wx = (px - fx).astype(np.float32)

    m_lo = int(max(0, -min(ay.min(), ax.min())))
    m_hi = int(max(0, max(ay.max(), ax.max()) + win - npad))
    wpad = npad + m_lo + m_hi

    pp = np.zeros((B, wpad, wpad), dtype=np.float32)
    pp[:, m_lo:m_lo + npad, m_lo:m_lo + npad] = padded

    c00 = ((1 - wy) * (1 - wx)).astype(np.float32)
    c10 = (wy * (1 - wx)).astype(np.float32)
    c01 = ((1 - wy) * wx).astype(np.float32)
    c11 = (wy * wx).astype(np.float32)

    dmat = np.zeros((128, 2 * chunk), dtype=np.float32)
    for m in range(chunk):
        dmat[m, m] = 1.0            # I
        dmat[m + 1, chunk + m] = 1.0  # S

    cfg = Cfg(bpc=bpc, n=n, wpad=wpad, chunk=chunk)
    nbig = cfg.nbig
    rem = cfg.rem
    P = chunk + 1
    PS = (rem[1] + 1) if rem else 1

    in_maps = []
    for cidx in range(n_cores):
        sl = slice(cidx * bpc, (cidx + 1) * bpc)
        A = (ay[sl] + m_lo).astype(np.int64)
        Bc = (ax[sl] + m_lo).astype(np.int64)
        base = np.arange(bpc, dtype=np.int64) * (wpad * wpad)
        # flat element offsets: big windowed DMA start, remainder-chunk start
        off_big = base + A * wpad + Bc
        if rem:
            off_small = off_big + (rem[0]) * wpad
        else:
            off_small = np.zeros_like(off_big)
        offs = np.empty((1, bpc * 2), dtype=np.int32)
        offs[0, 0::2] = off_big
        offs[0, 1::2] = off_small
        wmat = np.empty((bpc, 128, 4), dtype=np.float32)
        wmat[:, :, 0] = c00[sl][:, None]
        wmat[:, :, 1] = c10[sl][:, None]
        wmat[:, :, 2] = c01[sl][:, None]
        wmat[:, :, 3] = c11[sl][:, None]
        in_maps.append({
            "x": np.ascontiguousarray(pp[sl]),
            "offs": offs,
            "wmat": wmat,
            "dmat": dmat,
        })
    return cfg, in_maps


N_CORES = 8
CHUNK = 127
_nc_cache: dict = {}


def kernel(padded_obj: np.ndarray, positions: np.ndarray) -> np.ndarray:
    padded_obj = np.asarray(padded_obj)
    positions = np.asarray(positions)
    B, npad, _, C = padded_obj.shape
    cfg, in_maps = host_prep(
        padded_obj.reshape(B, npad, npad).astype(np.float32, copy=False),
        positions, N_CORES, CHUNK)

    nc = _nc_cache.get(cfg)
    if nc is None:
        nc = build_nc(cfg)
        _nc_cache[cfg] = nc

    res = run_bass_kernel_spmd(nc, in_maps, core_ids=list(range(N_CORES)))
    out = np.concatenate([r["y"] for r in res.results], axis=0)
    return out.reshape(B, cfg.n, cfg.n, 1).astype(np.float32, copy=False)



# revision 5
# speedup vs baseline: 7.4456x; 7.4456x over previous
"""Trainium2 Bass kernel for nn_ExtractPatchesPositionLayer.

Reference semantics: per image b, out[r, c] = bilinear sample of the
(522,522,1) padded object at (r + 5 + py_b, c + 5 + px_b), zero fill
outside -> (512,512,1). Per image the shift is constant, so floor/frac
give an integer window start (A,B) into a zero-margin-padded image plus
two 1-D blend weight pairs (1-wx,wx) and (1-wy,wy).

Layout trick: each SBUF partition p holds FOUR consecutive window rows
(4p..4p+3, plus spill into row 4p+4) as one contiguous DRAM run, so the
per-image window load is a single dynamic DMA of 128 contiguous ~5.2 KB
packets (instead of ~520 x 1 KB row packets), and the store is 128
contiguous 4 KB packets. Dynamic (register-offset) DMAs fan their
descriptors out across all 16 DMA engines; static-offset DMAs serialize
on one engine (this was the old kernel's bottleneck: all output packets
on engine 64 -> 1.42 ms).

Compute is pure elementwise fp16 (no PE/PSUM): with q = 0..4 row-slices
of the loaded tile,
    h[:, q, j] = (1-wx) * x[:, q, j] + wx * x[:, q, j+1]   (ACT mul + DVE stt)
    v[:, q, j] = (1-wy) * h[:, q, j] + wy * h[:, q+1, j]   (ACT mul + DVE stt)
where the q+1 vertical neighbor lives in the SAME partition (row spill),
so no cross-partition data movement at all.

Sharding: batch 256 -> 32 images x 8 cores, embarrassingly parallel.
"""

from dataclasses import dataclass

import numpy as np

import concourse.bacc as bacc
import concourse.bass as bass
import concourse.mybir as mybir
import concourse.tile as tile
from concourse.bass_utils import run_bass_kernel_spmd


@dataclass(frozen=True)
class Cfg:
    bpc: int   # images per core
    n: int     # output height/width (512)
    wpad: int  # padded input height/width (with zero margin)


def build_nc(cfg: Cfg) -> bass.Bass:
    BPC, N, W = cfg.bpc, cfg.n, cfg.wpad
    P = 128
    Q = N // P            # 4 rows per partition
    assert N == P * Q
    XW = Q * W + N + 2    # elements loaded per partition: 4 rows + spill
    HW5 = (Q + 1) * N     # 5 * 512 = h width incl spill row
    NQ = N                # output row width (512)
    f16 = mybir.dt.float16
    f32 = mybir.dt.float32
    i32 = mybir.dt.int32
    TOTX = BPC * W * W
    TOTY = BPC * N * N

    nc = bacc.Bacc("TRN2", target_bir_lowering=False, debug=False)
    x_d = nc.declare_dram_parameter("x", [BPC, W * W], f16, isOutput=False)
    offs_d = nc.declare_dram_parameter("offs", [1, BPC + 1], i32, isOutput=False)
    wmat_d = nc.declare_dram_parameter("wmat", [BPC, 128, 4], f32, isOutput=False)
    y_d = nc.declare_dram_parameter("y", [BPC, N * N], f16, isOutput=True)

    mult = mybir.AluOpType.mult
    add = mybir.AluOpType.add

    with tile.TileContext(nc) as tc:
        with (
            tc.tile_pool(name="const", bufs=1) as constp,
            tc.tile_pool(name="win", bufs=3) as winp,
            tc.tile_pool(name="hp", bufs=2) as hp,
            tc.tile_pool(name="vp", bufs=3) as vp,
        ):
            wmat_sb = constp.tile([128, BPC * 4], f32, tag="wmat")
            nc.sync.dma_start(
                wmat_sb[:].rearrange("p (i q) -> p i q", q=4),
                wmat_d[:, :, :].transpose([1, 0, 2]),
            )
            offs_sb = constp.tile([1, BPC + 1], i32, tag="offs")
            nc.sync.dma_start(offs_sb[:], offs_d[:, :])

            # input-offset register pool + one zero register for stores
            NREG = 8
            rin = [nc.alloc_register(mybir.EngineType.SP, f"rin{k}")
                   for k in range(NREG)]
            svin = [nc.snap(r, donate=True, min_val=0, max_val=TOTX - 1)
                    for r in rin]
            rz = nc.alloc_register(mybir.EngineType.SP, "rz")
            nc.sync.reg_load(rz, offs_sb[0:1, BPC:BPC + 1])
            svz = nc.snap(rz, donate=True, min_val=0, max_val=0)

            for i in range(BPC):
                k = i % NREG
                nc.sync.reg_load(rin[k], offs_sb[0:1, i:i + 1])
                x_t = winp.tile([128, (Q + 1) * W], f16, tag="x")
                nc.sync.dma_start(
                    x_t[:, 0:XW],
                    bass.AP(x_d, svin[k], [[Q * W, P], [1, XW]]),
                )
                x3 = x_t[:].rearrange("p (q w) -> p q w", w=W)
                in0 = x3[:, :, 0:NQ]
                in1 = x3[:, :, 1:NQ + 1]

                wx0 = wmat_sb[:, 4 * i + 0: 4 * i + 1]
                wx1 = wmat_sb[:, 4 * i + 1: 4 * i + 2]
                wy0 = wmat_sb[:, 4 * i + 2: 4 * i + 3]
                wy1 = wmat_sb[:, 4 * i + 3: 4 * i + 4]

                t1 = hp.tile([128, HW5], f16, tag="t1")
                h = hp.tile([128, HW5], f16, tag="h")
                t13 = t1[:].rearrange("p (q w) -> p q w", w=NQ)
                h3 = h[:].rearrange("p (q w) -> p q w", w=NQ)
                nc.scalar.mul(t13[:, :, :], in1, wx1)
                nc.vector.scalar_tensor_tensor(
                    h3[:, :, :], in0, wx0, t13[:, :, :], op0=mult, op1=add)

                t2 = vp.tile([128, Q * NQ], f16, tag="t2")
                v = vp.tile([128, Q * NQ], f16, tag="v")
                nc.scalar.mul(t2[:], h[:, NQ:HW5], wy1)
                nc.vector.scalar_tensor_tensor(
                    v[:], h[:, 0:Q * NQ], wy0, t2[:], op0=mult, op1=add)

                nc.sync.dma_start(
                    bass.AP(y_d, svz + i * N * N, [[Q * NQ, P], [1, Q * NQ]]),
                    v[:],
                )
    nc.compile()
    return nc


def host_prep(padded: np.ndarray, positions: np.ndarray, n_cores: int):
    """Shard + build metadata. padded: (B, npad, npad) f32, positions: (B, 2)."""
    B, npad, _ = padded.shape
    n = npad - 10
    bpc = B // n_cores
    win = n + 2  # need window cols B..B+513 and rows A..A+512 valid

    px = positions[:, 0].astype(np.float32)
    py = positions[:, 1].astype(np.float32)
    fy = np.floor(py)
    fx = np.floor(px)
    ay = (5 + fy).astype(np.int64)
    ax = (5 + fx).astype(np.int64)
    wy = (py - fy).astype(np.float32)
    wx = (px - fx).astype(np.float32)

    m_lo = int(max(0, -min(ay.min(), ax.min())))
    m_hi = int(max(0, max(ay.max(), ax.max()) + win - npad))
    wpad = npad + m_lo + m_hi

    pp = np.zeros((B, wpad, wpad), dtype=np.float16)
    pp[:, m_lo:m_lo + npad, m_lo:m_lo + npad] = padded.astype(np.float16)

    cfg = Cfg(bpc=bpc, n=n, wpad=wpad)

    in_maps = []
    for cidx in range(n_cores):
        sl = slice(cidx * bpc, (cidx + 1) * bpc)
        A = (ay[sl] + m_lo).astype(np.int64)
        Bc = (ax[sl] + m_lo).astype(np.int64)
        base = np.arange(bpc, dtype=np.int64) * (wpad * wpad)
        offs = np.zeros((1, bpc + 1), dtype=np.int32)
        offs[0, :bpc] = base + A * wpad + Bc
        wmat = np.empty((bpc, 128, 4), dtype=np.float32)
        wmat[:, :, 0] = (1.0 - wx[sl])[:, None]
        wmat[:, :, 1] = wx[sl][:, None]
        wmat[:, :, 2] = (1.0 - wy[sl])[:, None]
        wmat[:, :, 3] = wy[sl][:, None]
        in_maps.append({
            "x": np.ascontiguousarray(pp[sl].reshape(bpc, wpad * wpad)),
            "offs": offs,
            "wmat": wmat,
        })
    return cfg, in_maps


N_CORES = 8
_nc_cache: dict = {}


def kernel(padded_obj: np.ndarray, positions: np.ndarray) -> np.ndarray:
    padded_obj = np.asarray(padded_obj)
    positions = np.asarray(positions)
    B, npad, _, C = padded_obj.shape
    cfg, in_maps = host_prep(
        padded_obj.reshape(B, npad, npad).astype(np.float32, copy=False),
        positions, N_CORES)

    nc = _nc_cache.get(cfg)
    if nc is None:
        nc = build_nc(cfg)
        _nc_cache[cfg] = nc

    res = run_bass_kernel_spmd(nc, in_maps, core_ids=list(range(N_CORES)))
    out = np.concatenate([r["y"] for r in res.results], axis=0)
    return out.reshape(B, cfg.n, cfg.n, 1).astype(np.float32)


# revision 7
# speedup vs baseline: 8.7250x; 1.1718x over previous
"""Trainium2 Bass kernel for nn_ExtractPatchesPositionLayer.

Reference semantics: per image b, out[r, c] = bilinear sample of the
(522,522,1) padded object at (r + 5 + py_b, c + 5 + px_b), zero fill
outside -> (512,512,1). Per image the shift is constant, so floor/frac
give an integer window start (A,B) into a zero-margin-padded image plus
two 1-D blend weight pairs (1-wx,wx) and (1-wy,wy).

Layout trick: each SBUF partition p holds FOUR consecutive window rows
(4p..4p+3, plus spill into row 4p+4) as one contiguous DRAM run, so the
per-image window load is a single dynamic DMA of 128 contiguous ~5.2 KB
packets (instead of ~520 x 1 KB row packets), and the store is 128
contiguous 4 KB packets. Dynamic (register-offset) DMAs fan their
descriptors out across all 16 DMA engines; static-offset DMAs serialize
on one engine (this was the old kernel's bottleneck: all output packets
on engine 64 -> 1.42 ms).

Compute is pure elementwise fp16 (no PE/PSUM): with q = 0..4 row-slices
of the loaded tile,
    h[:, q, j] = (1-wx) * x[:, q, j] + wx * x[:, q, j+1]   (ACT mul + DVE stt)
    v[:, q, j] = (1-wy) * h[:, q, j] + wy * h[:, q+1, j]   (ACT mul + DVE stt)
where the q+1 vertical neighbor lives in the SAME partition (row spill),
so no cross-partition data movement at all.

Sharding: batch 256 -> 32 images x 8 cores, embarrassingly parallel.
"""

from dataclasses import dataclass

import numpy as np

import concourse.bacc as bacc
import concourse.bass as bass
import concourse.mybir as mybir
import concourse.tile as tile
from concourse.bass_utils import run_bass_kernel_spmd


@dataclass(frozen=True)
class Cfg:
    bpc: int   # images per core
    n: int     # output height/width (512)
    wpad: int  # padded input height/width (with zero margin)


def build_nc(cfg: Cfg) -> bass.Bass:
    BPC, N, W = cfg.bpc, cfg.n, cfg.wpad
    P = 128
    Q = N // P            # 4 rows per partition
    assert N == P * Q
    XW = Q * W + N + 2    # elements loaded per partition: 4 rows + spill
    HW5 = (Q + 1) * N     # 5 * 512 = h width incl spill row
    NQ = N                # output row width (512)
    f16 = mybir.dt.float16
    f32 = mybir.dt.float32
    i32 = mybir.dt.int32
    TOTX = BPC * W * W
    TOTY = BPC * N * N

    nc = bacc.Bacc("TRN2", target_bir_lowering=False, debug=False)
    x_d = nc.declare_dram_parameter("x", [BPC, W * W], f16, isOutput=False)
    offs_d = nc.declare_dram_parameter("offs", [1, BPC + 1], i32, isOutput=False)
    wmat_d = nc.declare_dram_parameter("wmat", [BPC, 128, 4], f32, isOutput=False)
    y_d = nc.declare_dram_parameter("y", [BPC, N * N], f16, isOutput=True)

    mult = mybir.AluOpType.mult
    add = mybir.AluOpType.add

    with tile.TileContext(nc) as tc:
        with (
            tc.tile_pool(name="const", bufs=1) as constp,
            tc.tile_pool(name="win", bufs=3) as winp,
            tc.tile_pool(name="hp", bufs=2) as hp,
            tc.tile_pool(name="vp", bufs=3) as vp,
        ):
            wmat_sb = constp.tile([128, BPC * 4], f32, tag="wmat")
            nc.sync.dma_start(
                wmat_sb[:].rearrange("p (i q) -> p i q", q=4),
                wmat_d[:, :, :].transpose([1, 0, 2]),
            )
            offs_sb = constp.tile([1, BPC + 1], i32, tag="offs")
            nc.sync.dma_start(offs_sb[:], offs_d[:, :])

            # input-offset register pool + one zero register for stores
            NREG = 8
            rin = [nc.alloc_register(mybir.EngineType.SP, f"rin{k}")
                   for k in range(NREG)]
            svin = [nc.snap(r, donate=True, min_val=0, max_val=TOTX - 1)
                    for r in rin]
            rz = nc.alloc_register(mybir.EngineType.SP, "rz")
            nc.sync.reg_load(rz, offs_sb[0:1, BPC:BPC + 1])
            svz = nc.snap(rz, donate=True, min_val=0, max_val=0)

            for i in range(BPC):
                k = i % NREG
                nc.sync.reg_load(rin[k], offs_sb[0:1, i:i + 1])
                x_t = winp.tile([128, (Q + 1) * W], f16, tag="x")
                nc.sync.dma_start(
                    x_t[:, 0:XW],
                    bass.AP(x_d, svin[k], [[Q * W, P], [1, XW]]),
                )
                x3 = x_t[:].rearrange("p (q w) -> p q w", w=W)

                wx0 = wmat_sb[:, 4 * i + 0: 4 * i + 1]
                wx1 = wmat_sb[:, 4 * i + 1: 4 * i + 2]
                wy0 = wmat_sb[:, 4 * i + 2: 4 * i + 3]
                wy1 = wmat_sb[:, 4 * i + 3: 4 * i + 4]

                # vertical blend first (on NQ+2-wide strips), then horizontal.
                # tensor_scalar (4x DVE mode) + tensor_tensor (2x) beat
                # scalar_tensor_tensor (no fast mode, 1x).
                VW = NQ + 2
                mv1 = hp.tile([128, Q * VW], f16, tag="mv1")
                mv2 = hp.tile([128, Q * VW], f16, tag="mv2")
                vv = hp.tile([128, Q * VW], f16, tag="vv")
                mv13 = mv1[:].rearrange("p (q w) -> p q w", w=VW)
                mv23 = mv2[:].rearrange("p (q w) -> p q w", w=VW)
                vv3 = vv[:].rearrange("p (q w) -> p q w", w=VW)
                nc.scalar.mul(mv23[:, :, :], x3[:, 1:Q + 1, 0:VW], wy1)
                nc.vector.tensor_scalar(
                    mv13[:, :, :], x3[:, 0:Q, 0:VW], wy0, None, op0=mult)
                nc.vector.tensor_tensor(
                    vv3[:, :, :], mv13[:, :, :], mv23[:, :, :], op=add)

                mh1 = vp.tile([128, Q * NQ], f16, tag="mh1")
                mh2 = vp.tile([128, Q * NQ], f16, tag="mh2")
                v = vp.tile([128, Q * NQ], f16, tag="v")
                mh13 = mh1[:].rearrange("p (q w) -> p q w", w=NQ)
                mh23 = mh2[:].rearrange("p (q w) -> p q w", w=NQ)
                nc.vector.tensor_scalar(
                    mh13[:, :, :], vv3[:, :, 0:NQ], wx0, None, op0=mult)
                nc.vector.tensor_scalar(
                    mh23[:, :, :], vv3[:, :, 1:NQ + 1], wx1, None, op0=mult)
                nc.vector.tensor_tensor(v[:], mh1[:], mh2[:], op=add)

                nc.sync.dma_start(
                    bass.AP(y_d, svz + i * N * N, [[Q * NQ, P], [1, Q * NQ]]),
                    v[:],
                )
    nc.compile()
    return nc


def host_prep(padded: np.ndarray, positions: np.ndarray, n_cores: int):
    """Shard + build metadata. padded: (B, npad, npad) f32, positions: (B, 2)."""
    B, npad, _ = padded.shape
    n = npad - 10
    bpc = B // n_cores
    win = n + 2  # need window cols B..B+513 and rows A..A+512 valid

    px = positions[:, 0].astype(np.float32)
    py = positions[:, 1].astype(np.float32)
    fy = np.floor(py)
    fx = np.floor(px)
    ay = (5 + fy).astype(np.int64)
    ax = (5 + fx).astype(np.int64)
    wy = (py - fy).astype(np.float32)
    wx = (px - fx).astype(np.float32)

    m_lo = int(max(0, -min(ay.min(), ax.min())))
    m_hi = int(max(0, max(ay.max(), ax.max()) + win - npad))
    # round the padded width up to a multiple of 8 so the 4-row partition
    # stride (8*wpad bytes in fp16) is 64B-aligned for the DMA engines
    wpad = -(-(npad + m_lo + m_hi) // 8) * 8

    pp = np.zeros((B, wpad, wpad), dtype=np.float16)
    pp[:, m_lo:m_lo + npad, m_lo:m_lo + npad] = padded.astype(np.float16)

    cfg = Cfg(bpc=bpc, n=n, wpad=wpad)

    in_maps = []
    for cidx in range(n_cores):
        sl = slice(cidx * bpc, (cidx + 1) * bpc)
        A = (ay[sl] + m_lo).astype(np.int64)
        Bc = (ax[sl] + m_lo).astype(np.int64)
        base = np.arange(bpc, dtype=np.int64) * (wpad * wpad)
        offs = np.zeros((1, bpc + 1), dtype=np.int32)
        offs[0, :bpc] = base + A * wpad + Bc
        wmat = np.empty((bpc, 128, 4), dtype=np.float32)
        wmat[:, :, 0] = (1.0 - wx[sl])[:, None]
        wmat[:, :, 1] = wx[sl][:, None]
        wmat[:, :, 2] = (1.0 - wy[sl])[:, None]
        wmat[:, :, 3] = wy[sl][:, None]
        in_maps.append({
            "x": np.ascontiguousarray(pp[sl].reshape(bpc, wpad * wpad)),
            "offs": offs,
            "wmat": wmat,
        })
    return cfg, in_maps


N_CORES = 8
_nc_cache: dict = {}


def kernel(padded_obj: np.ndarray, positions: np.ndarray) -> np.ndarray:
    padded_obj = np.asarray(padded_obj)
    positions = np.asarray(positions)
    B, npad, _, C = padded_obj.shape
    cfg, in_maps = host_prep(
        padded_obj.reshape(B, npad, npad).astype(np.float32, copy=False),
        positions, N_CORES)

    nc = _nc_cache.get(cfg)
    if nc is None:
        nc = build_nc(cfg)
        _nc_cache[cfg] = nc

    res = run_bass_kernel_spmd(nc, in_maps, core_ids=list(range(N_CORES)))
    out = np.concatenate([r["y"] for r in res.results], axis=0)
    return out.reshape(B, cfg.n, cfg.n, 1).astype(np.float32)
